# revision 28
# baseline (speedup 1.0000x reference)
"""Trainium2 Bass kernel for the AlgelogicNetwork problem.

Math (per batch element b, all rule params tiny):
  s[w,l]   : 9 WM slots, 2-dim tokens (state row reshaped [9,2])
  tm[m,w]  = K0'[m] + sum_l Q[m,l]*(s[w,l]+r[m,l])^2     (completed square)
  minm[m]  = min_w tm[m,w];  onehot[m,w] = (tm==minm)
  sb[m,l]  = sum_w onehot[m,w]*s[w,l]                     (best-slot gather)
  conc2x[m,l] = sum_k 2*C[m,l,k]*sb[m,k] + 2*d[m,l]       (2x conclusion)
  E[m,w]   = conc2x[m,:].s[w,:] - |conc2x/2|^2 - |s[w]|^2 - minm[m] - K0'[m]
           = -|conc-s[w]|^2 - minmatch[m]   (so exp(E) = score*confidence)
  out[w]   = sum_m exp(E[m,w])

Sharding: pure data parallel, batch 262144 split 8 ways (32768/core).
Layout: batch in partitions; each SBUF tile holds T batch-groups of 128.
"""

import os
import sys

import numpy as np

if "/opt/trn_rl_repo" not in sys.path:
    sys.path.insert(0, "/opt/trn_rl_repo")

import concourse.bacc as bacc
import concourse.bass as bass
import concourse.tile as tile
from concourse import mybir
from concourse.bass_utils import run_bass_kernel_spmd

F32 = mybir.dt.float32
OP = mybir.AluOpType

M, J, NI, L, W = 16, 2, 3, 2, 9
B = 262144
NCORES = 8
CB = B // NCORES  # 32768 per core
P = 128
T = 16  # batch groups per partition per tile
TILE_B = P * T  # 1024
NTILES = CB // TILE_B  # 32

TRACE = False
LAST_EXEC_NS = None


def _tables(constants, gammas, head_W, head_b, tail_W, tail_b):
    """Host-side folding of the tiny rule params into kernel coefficient tables."""
    f8 = np.float64
    g = 1.0 / (1.0 + np.exp(-gammas[:, :J].astype(f8)))  # [M,J,L]
    c = constants[:, :J].astype(f8)
    omg = 1.0 - g
    Q = omg.sum(1)  # [M,L]
    Pc = (omg * c).sum(1)  # [M,L]
    K2 = (omg * c * c).sum(1)  # [M,L]
    r = -Pc / Q  # [M,L]
    K0p = (K2 - Pc * Pc / Q).sum(1)  # [M]
    A = np.einsum("mjil,mjl->mil", head_W.astype(f8), g)  # [M,I,L]
    bias = np.einsum("mj,mji->mi", g.sum(2), head_b.astype(f8))  # [M,I]
    C = np.einsum("mli,mik->mlk", tail_W.astype(f8), A)  # [M,L,L]
    d = np.einsum("mli,mi->ml", tail_W.astype(f8), bias) + tail_b.astype(f8)  # [M,L]
    C2 = 2.0 * C
    d2 = 2.0 * d

    # rtab: [M,W,L] broadcast of r over w, flattened [288]
    rexp = np.broadcast_to(r[:, None, :], (M, W, L)).reshape(M * W * L)
    # ctab: 9 16-wide tables: q0,q1,k0p,c2_00,c2_01,c2_10,c2_11,d2_0,d2_1
    ctab = np.concatenate(
        [
            Q[:, 0], Q[:, 1], K0p,
            C2[:, 0, 0], C2[:, 0, 1], C2[:, 1, 0], C2[:, 1, 1],
            d2[:, 0], d2[:, 1],
        ]
    )
    rtab128 = np.ascontiguousarray(
        np.broadcast_to(rexp[None].astype(np.float32), (P, rexp.size))
    )
    ctab128 = np.ascontiguousarray(
        np.broadcast_to(ctab[None].astype(np.float32), (P, ctab.size))
    )
    return rtab128, ctab128


def _build():
    nc = bacc.Bacc()
    state = nc.declare_dram_parameter("state", [CB, W * L], F32, isOutput=False)
    tabs = nc.declare_dram_parameter(
        "tabs", [P, M * W * L + 9 * M], F32, isOutput=False
    )
    out = nc.declare_dram_parameter("out", [CB, W], F32, isOutput=True)

    state_t = state[:].rearrange("(n p t) d -> n p (t d)", p=P, t=T)  # [NTILES,128,T*18]
    out_t = out[:].rearrange("(n p t) w -> n p (t w)", p=P, t=T)  # [NTILES,128,T*9]

    from contextlib import ExitStack

    with tile.TileContext(nc) as tc, ExitStack() as ctx:
        cpool = ctx.enter_context(tc.tile_pool(name="consts", bufs=1))
        tabs_t = cpool.tile([P, M * W * L + 9 * M], F32)
        nc.sync.dma_start(tabs_t[:], tabs[:])
        rtab_t = tabs_t[:, : M * W * L]
        ctab_t = tabs_t[:, M * W * L :]
        # gate: absorb the const-DMA wait on DVE once, so per-tile TensorTensor
        # ops never need a second sync-wait slot (TT encoding has only one)
        gate = cpool.tile([P, 1], F32)
        nc.vector.tensor_copy(gate[:], tabs_t[:, :1])
        # explicit zero bias column: keeps Activations off the shared const-ap
        # tile, whose extra semaphore wait overflows the 2-slot AC encoding
        zero_col = cpool.tile([P, 1], F32)
        nc.vector.tensor_tensor(zero_col[:], gate[:], gate[:], OP.subtract)
        one_col = cpool.tile([P, 1], F32)
        nc.vector.tensor_scalar(one_col[:], zero_col[:], 1.0, None, OP.add)

        def ct(i):  # i-th [128,16] const table
            return ctab_t[:, M * i : M * (i + 1)]

        def bc_m(ap_16):  # [128,16] -> [128,T,16] broadcast over t
            return ap_16.unsqueeze(1).broadcast_to([P, T, M])

        def bc_mw(ap_16):  # [128,16] -> [128,T,16,9]
            return ap_16.unsqueeze(1).unsqueeze(3).broadcast_to([P, T, M, W])

        sp = ctx.enter_context(tc.tile_pool(name="s", bufs=3))
        zp = ctx.enter_context(tc.tile_pool(name="z", bufs=1))
        z2p = ctx.enter_context(tc.tile_pool(name="z2", bufs=2))
        wp = ctx.enter_context(tc.tile_pool(name="work", bufs=2))
        # one slot per tile: output tiles are never reused, so the DVE reduce
        # that writes them never waits on an out-DMA queue release (each
        # compute instruction only has one sync-wait slot)
        op_ = ctx.enter_context(tc.tile_pool(name="outp", bufs=8))

        for n in range(NTILES):
            s_t = sp.tile([P, T * W * L], F32, tag="s")
            nc.sync.dma_start(s_t[:], state_t[n])
            s_tw = s_t[:].rearrange("p (t w l) -> p t w l", t=T, w=W)  # views
            s_tmd = (
                s_t[:].rearrange("p (t d) -> p t d", t=T)
                .unsqueeze(2)
                .broadcast_to([P, T, M, W * L])
            )

            # --- tm[m,w] = sum_l Q_l*(s_wl + r_ml)^2  (K0' folded later) ---
            z = zp.tile([P, T * M * W * L], F32, tag="z")
            z_v = z[:].rearrange("p (t m d) -> p t m d", t=T, m=M)
            r_bc = (
                rtab_t.rearrange("p (m d) -> p m d", m=M)
                .unsqueeze(1)
                .broadcast_to([P, T, M, W * L])
            )
            nc.vector.tensor_tensor(z_v, s_tmd, r_bc, OP.add)
            z2 = z2p.tile([P, T * M * W * L], F32, tag="z2")
            nc.scalar.activation(z2[:], z[:], mybir.ActivationFunctionType.Square, bias=zero_col[:])
            z2_v = z2[:].rearrange("p (t m w l) -> p t m w l", t=T, m=M, w=W)
            m1 = wp.tile([P, T * M * W], F32, tag="m1")
            m1_v = m1[:].rearrange("p (t m w) -> p t m w", t=T, m=M)
            nc.gpsimd.tensor_tensor(m1_v, z2_v[:, :, :, :, 0], bc_mw(ct(0)), OP.mult)
            m2 = wp.tile([P, T * M * W], F32, tag="m2")
            m2_v = m2[:].rearrange("p (t m w) -> p t m w", t=T, m=M)
            nc.gpsimd.tensor_tensor(m2_v, z2_v[:, :, :, :, 1], bc_mw(ct(1)), OP.mult)
            tm = wp.tile([P, T * M * W], F32, tag="tm")
            nc.vector.tensor_tensor(tm[:], m1[:], m2[:], OP.add)
            tm_v = tm[:].rearrange("p (t m w) -> p t m w", t=T, m=M)

            # contiguous per-component slot values: avoids stride-2 operand
            # APs on the four big DVE products below (measured ~40% slower)
            sx = wp.tile([P, T * 2 * W], F32, tag="sx")  # [t,k,w]
            sx_v = sx[:].rearrange("p (t k w) -> p t k w", t=T, k=2)
            for k in range(2):
                nc.scalar.copy(sx_v[:, :, k, :], s_tw[:, :, :, k])

            # |s_w|^2 and its exp factor, early so ACT finishes them long
            # before the tile-end multiply needs exps2
            ss = wp.tile([P, T * W * L], F32, tag="ss")
            nc.scalar.activation(ss[:], s_t[:], mybir.ActivationFunctionType.Square, bias=zero_col[:])
            ss_v = ss[:].rearrange("p (t w l) -> p t w l", t=T, w=W)
            s2s = wp.tile([P, T * W], F32, tag="s2s")
            s2s_v = s2s[:].rearrange("p (t w) -> p t w", t=T)
            nc.vector.tensor_tensor(
                s2s_v, ss_v[:, :, :, 0], ss_v[:, :, :, 1], OP.add
            )

            # --- min over w, onehot, best-slot gather ---
            minv = wp.tile([P, T * M], F32, tag="minv")
            minv_v = minv[:].rearrange("p (t m) -> p t m", t=T)
            nc.vector.tensor_reduce(minv_v, tm_v, mybir.AxisListType.X, OP.min)
            oh = wp.tile([P, T * M * W], F32, tag="oh")
            oh_v = oh[:].rearrange("p (t m w) -> p t m w", t=T, m=M)
            minv_bc = minv_v.unsqueeze(3).broadcast_to([P, T, M, W])
            nc.vector.tensor_tensor(oh_v, tm_v, minv_bc, OP.is_le)

            sb = wp.tile([P, T * 2 * M], F32, tag="sb")  # [t,k,m]
            sb_v = sb[:].rearrange("p (t k m) -> p t k m", t=T, k=2)
            for k in range(2):
                pk = wp.tile([P, T * M * W], F32, tag=f"pk{k}")
                pk_v = pk[:].rearrange("p (t m w) -> p t m w", t=T, m=M)
                s_k = sx_v[:, :, k, :].unsqueeze(2).broadcast_to([P, T, M, W])
                nc.vector.tensor_tensor(pk_v, oh_v, s_k, OP.mult)
                nc.vector.tensor_reduce(
                    sb_v[:, :, k, :], pk_v, mybir.AxisListType.X, OP.add
                )

            # --- conc2x[t,l,m] = sum_k C2[l,k]*sb[k] + d2[l] ---
            conc = wp.tile([P, T * 2 * M], F32, tag="conc")  # [t,l,m]
            conc_v = conc[:].rearrange("p (t l m) -> p t l m", t=T, l=2)
            for l in range(2):
                x0 = wp.tile([P, T * M], F32, tag="x0")
                x0_v = x0[:].rearrange("p (t m) -> p t m", t=T)
                nc.vector.tensor_tensor(
                    x0_v, sb_v[:, :, 0, :], bc_m(ct(3 + 2 * l)), OP.mult
                )
                y0 = wp.tile([P, T * M], F32, tag="y0")
                y0_v = y0[:].rearrange("p (t m) -> p t m", t=T)
                nc.vector.tensor_tensor(
                    y0_v, sb_v[:, :, 1, :], bc_m(ct(4 + 2 * l)), OP.mult
                )
                nc.vector.tensor_tensor(x0_v, x0_v, y0_v, OP.add)
                nc.vector.tensor_tensor(
                    conc_v[:, :, l, :], x0_v, bc_m(ct(7 + l)), OP.add
                )

            # --- A3[t,m] = |conc|^2 + minm + K0' ---
            cq = wp.tile([P, T * 2 * M], F32, tag="cq")
            nc.scalar.activation(
                cq[:], conc[:], mybir.ActivationFunctionType.Square,
                bias=zero_col[:], scale=0.5,
            )
            cq_v = cq[:].rearrange("p (t l m) -> p t l m", t=T, l=2)
            a3 = wp.tile([P, T * M], F32, tag="a3")
            a3_v = a3[:].rearrange("p (t m) -> p t m", t=T)
            nc.vector.tensor_tensor(a3_v, cq_v[:, :, 0, :], cq_v[:, :, 1, :], OP.add)
            nc.vector.tensor_tensor(a3[:], a3[:], minv[:], OP.add)
            nc.vector.tensor_tensor(a3_v, a3_v, bc_m(ct(2)), OP.add)


            # --- E[t,w,m] = conc2x.s - A3 - |s|^2 ;  score = exp(E) ---
            m1e = wp.tile([P, T * W * M], F32, tag="m1")
            m1e_v = m1e[:].rearrange("p (t w m) -> p t w m", t=T, w=W)
            conc0_bc = conc_v[:, :, 0, :].unsqueeze(2).broadcast_to([P, T, W, M])
            s0_bc = sx_v[:, :, 0, :].unsqueeze(3).broadcast_to([P, T, W, M])
            nc.vector.tensor_tensor(m1e_v, conc0_bc, s0_bc, OP.mult)
            m2e = wp.tile([P, T * W * M], F32, tag="m2")
            m2e_v = m2e[:].rearrange("p (t w m) -> p t w m", t=T, w=W)
            conc1_bc = conc_v[:, :, 1, :].unsqueeze(2).broadcast_to([P, T, W, M])
            s1_bc = sx_v[:, :, 1, :].unsqueeze(3).broadcast_to([P, T, W, M])
            nc.vector.tensor_tensor(m2e_v, conc1_bc, s1_bc, OP.mult)

            a3_bc = a3_v.unsqueeze(2).broadcast_to([P, T, W, M])
            nc.vector.tensor_tensor(m1e_v, m1e_v, a3_bc, OP.subtract)
            s2s_bc = s2s_v.unsqueeze(3).broadcast_to([P, T, W, M])
            nc.vector.tensor_tensor(m2e_v, m2e_v, s2s_bc, OP.subtract)
            ee = wp.tile([P, T * W * M], F32, tag="tm")
            nc.vector.tensor_tensor(ee[:], m1e[:], m2e[:], OP.add)
            sc = wp.tile([P, T * W * M], F32, tag="oh")
            nc.scalar.activation(sc[:], ee[:], mybir.ActivationFunctionType.Exp, bias=zero_col[:])

            # --- out[t,w] = sum_m score ---
            ov = op_.tile([P, T * W], F32, tag="ov")
            ov_v = ov[:].rearrange("p (t w) -> p t w", t=T)
            sc_v = sc[:].rearrange("p (t w m) -> p t w m", t=T, w=W)
            nc.vector.tensor_reduce(ov_v, sc_v, mybir.AxisListType.X, OP.add)
            nc.sync.dma_start(out_t[n], ov[:])

    nc.compile()
    return nc


_NC_CACHE = None


def kernel(state, constants, gammas, head_W, head_b, tail_W, tail_b):
    global LAST_EXEC_NS, _NC_CACHE
    state = np.ascontiguousarray(np.asarray(state, dtype=np.float32))
    rtab128, ctab128 = _tables(
        np.asarray(constants), np.asarray(gammas), np.asarray(head_W),
        np.asarray(head_b), np.asarray(tail_W), np.asarray(tail_b),
    )
    if _NC_CACHE is None:
        _NC_CACHE = _build()
    nc = _NC_CACHE
    tabs128 = np.ascontiguousarray(np.concatenate([rtab128, ctab128], axis=1))
    in_maps = []
    for i in range(NCORES):
        in_maps.append(
            {
                "state": state[i * CB : (i + 1) * CB],
                "tabs": tabs128,
            }
        )
    res = run_bass_kernel_spmd(nc, in_maps, core_ids=list(range(NCORES)), trace=TRACE)
    LAST_EXEC_NS = res.exec_time_ns
    out = np.concatenate([res.results[i]["out"] for i in range(NCORES)], axis=0)
    return out


# revision 29
# speedup vs baseline: 1.1991x; 1.1991x over previous
"""Trainium2 Bass kernel for the AlgelogicNetwork problem.

Math (per batch element b, all rule params tiny):
  s[w,l]   : 9 WM slots, 2-dim tokens (state row reshaped [9,2])
  tm[m,w]  = K0'[m] + sum_l Q[m,l]*(s[w,l]+r[m,l])^2     (completed square)
  minm[m]  = min_w tm[m,w];  onehot[m,w] = (tm==minm)
  sb[m,l]  = sum_w onehot[m,w]*s[w,l]                     (best-slot gather)
  conc2x[m,l] = sum_k 2*C[m,l,k]*sb[m,k] + 2*d[m,l]       (2x conclusion)
  E[m,w]   = conc2x[m,:].s[w,:] - |conc2x/2|^2 - |s[w]|^2 - minm[m] - K0'[m]
           = -|conc-s[w]|^2 - minmatch[m]   (so exp(E) = score*confidence)
  out[w]   = sum_m exp(E[m,w])

Sharding: pure data parallel, batch 262144 split 8 ways (32768/core).
Layout: batch in partitions; each SBUF tile holds T batch-groups of 128.
"""

import os
import sys

import numpy as np

if "/opt/trn_rl_repo" not in sys.path:
    sys.path.insert(0, "/opt/trn_rl_repo")

import concourse.bacc as bacc
import concourse.bass as bass
import concourse.tile as tile
from concourse import mybir
from concourse.bass_utils import run_bass_kernel_spmd

F32 = mybir.dt.float32
OP = mybir.AluOpType

M, J, NI, L, W = 16, 2, 3, 2, 9
B = 262144
NCORES = 8
CB = B // NCORES  # 32768 per core
P = 128
T = 16  # batch groups per partition per tile
TILE_B = P * T  # 1024
NTILES = CB // TILE_B  # 32

TRACE = False
LAST_EXEC_NS = None


def _tables(constants, gammas, head_W, head_b, tail_W, tail_b):
    """Host-side folding of the tiny rule params into kernel coefficient tables."""
    f8 = np.float64
    g = 1.0 / (1.0 + np.exp(-gammas[:, :J].astype(f8)))  # [M,J,L]
    c = constants[:, :J].astype(f8)
    omg = 1.0 - g
    Q = omg.sum(1)  # [M,L]
    Pc = (omg * c).sum(1)  # [M,L]
    K2 = (omg * c * c).sum(1)  # [M,L]
    r = -Pc / Q  # [M,L]
    K0p = (K2 - Pc * Pc / Q).sum(1)  # [M]
    A = np.einsum("mjil,mjl->mil", head_W.astype(f8), g)  # [M,I,L]
    bias = np.einsum("mj,mji->mi", g.sum(2), head_b.astype(f8))  # [M,I]
    C = np.einsum("mli,mik->mlk", tail_W.astype(f8), A)  # [M,L,L]
    d = np.einsum("mli,mi->ml", tail_W.astype(f8), bias) + tail_b.astype(f8)  # [M,L]
    C2 = 2.0 * C
    d2 = 2.0 * d

    # rtab: [M,W,L] broadcast of r over w, flattened [288]
    rexp = np.broadcast_to(r[:, None, :], (M, W, L)).reshape(M * W * L)
    # ctab: 9 16-wide tables: q0,q1,k0p,c2_00,c2_01,c2_10,c2_11,d2_0,d2_1
    ctab = np.concatenate(
        [
            Q[:, 0], Q[:, 1], K0p,
            C2[:, 0, 0], C2[:, 0, 1], C2[:, 1, 0], C2[:, 1, 1],
            d2[:, 0], d2[:, 1],
        ]
    )
    rtab128 = np.ascontiguousarray(
        np.broadcast_to(rexp[None].astype(np.float32), (P, rexp.size))
    )
    ctab128 = np.ascontiguousarray(
        np.broadcast_to(ctab[None].astype(np.float32), (P, ctab.size))
    )
    return rtab128, ctab128


def _build():
    nc = bacc.Bacc()
    state = nc.declare_dram_parameter("state", [CB, W * L], F32, isOutput=False)
    tabs = nc.declare_dram_parameter(
        "tabs", [P, M * W * L + 9 * M], F32, isOutput=False
    )
    out = nc.declare_dram_parameter("out", [CB, W], F32, isOutput=True)

    state_t = state[:].rearrange("(n p t) d -> n p (t d)", p=P, t=T)  # [NTILES,128,T*18]
    out_t = out[:].rearrange("(n p t) w -> n p (t w)", p=P, t=T)  # [NTILES,128,T*9]

    from contextlib import ExitStack

    with tile.TileContext(nc) as tc, ExitStack() as ctx:
        cpool = ctx.enter_context(tc.tile_pool(name="consts", bufs=1))
        tabs_t = cpool.tile([P, M * W * L + 9 * M], F32)
        nc.sync.dma_start(tabs_t[:], tabs[:])
        rtab_t = tabs_t[:, : M * W * L]
        ctab_t = tabs_t[:, M * W * L :]
        # gate: absorb the const-DMA wait on DVE once, so per-tile TensorTensor
        # ops never need a second sync-wait slot (TT encoding has only one)
        gate = cpool.tile([P, 1], F32)
        nc.vector.tensor_copy(gate[:], tabs_t[:, :1])
        # explicit zero bias column: keeps Activations off the shared const-ap
        # tile, whose extra semaphore wait overflows the 2-slot AC encoding
        zero_col = cpool.tile([P, 1], F32)
        nc.vector.tensor_tensor(zero_col[:], gate[:], gate[:], OP.subtract)

        def ct(i):  # i-th [128,16] const table
            return ctab_t[:, M * i : M * (i + 1)]

        def bc_m(ap_16):  # [128,16] -> [128,T,16] broadcast over t
            return ap_16.unsqueeze(1).broadcast_to([P, T, M])

        def bc_mw(ap_16):  # [128,16] -> [128,T,16,9]
            return ap_16.unsqueeze(1).unsqueeze(3).broadcast_to([P, T, M, W])

        sp = ctx.enter_context(tc.tile_pool(name="s", bufs=3))
        zp = ctx.enter_context(tc.tile_pool(name="z", bufs=1))
        z2p = ctx.enter_context(tc.tile_pool(name="z2", bufs=2))
        wp = ctx.enter_context(tc.tile_pool(name="work", bufs=2))
        # one slot per tile: output tiles are never reused, so the DVE reduce
        # that writes them never waits on an out-DMA queue release (each
        # compute instruction only has one sync-wait slot)
        op_ = ctx.enter_context(tc.tile_pool(name="outp", bufs=NTILES))

        for n in range(NTILES):
            s_t = sp.tile([P, T * W * L], F32, tag="s")
            nc.sync.dma_start(s_t[:], state_t[n])
            s_tw = s_t[:].rearrange("p (t w l) -> p t w l", t=T, w=W)  # views
            s_tmd = (
                s_t[:].rearrange("p (t d) -> p t d", t=T)
                .unsqueeze(2)
                .broadcast_to([P, T, M, W * L])
            )

            # --- tm[m,w] = sum_l Q_l*(s_wl + r_ml)^2  (K0' folded later) ---
            z = zp.tile([P, T * M * W * L], F32, tag="z")
            z_v = z[:].rearrange("p (t m d) -> p t m d", t=T, m=M)
            r_bc = (
                rtab_t.rearrange("p (m d) -> p m d", m=M)
                .unsqueeze(1)
                .broadcast_to([P, T, M, W * L])
            )
            nc.vector.tensor_tensor(z_v, s_tmd, r_bc, OP.add)
            z2 = z2p.tile([P, T * M * W * L], F32, tag="z2")
            nc.scalar.activation(z2[:], z[:], mybir.ActivationFunctionType.Square, bias=zero_col[:])
            z2_v = z2[:].rearrange("p (t m w l) -> p t m w l", t=T, m=M, w=W)
            m1 = wp.tile([P, T * M * W], F32, tag="m1")
            m1_v = m1[:].rearrange("p (t m w) -> p t m w", t=T, m=M)
            nc.gpsimd.tensor_tensor(m1_v, z2_v[:, :, :, :, 0], bc_mw(ct(0)), OP.mult)
            m2 = wp.tile([P, T * M * W], F32, tag="m2")
            m2_v = m2[:].rearrange("p (t m w) -> p t m w", t=T, m=M)
            nc.gpsimd.tensor_tensor(m2_v, z2_v[:, :, :, :, 1], bc_mw(ct(1)), OP.mult)
            tm = wp.tile([P, T * M * W], F32, tag="tm")
            nc.vector.tensor_tensor(tm[:], m1[:], m2[:], OP.add)
            tm_v = tm[:].rearrange("p (t m w) -> p t m w", t=T, m=M)

            # contiguous per-component slot values: avoids stride-2 operand
            # APs on the four big DVE products below (measured ~40% slower)
            sx = wp.tile([P, T * 2 * W], F32, tag="sx")  # [t,k,w]
            sx_v = sx[:].rearrange("p (t k w) -> p t k w", t=T, k=2)
            for k in range(2):
                nc.scalar.copy(sx_v[:, :, k, :], s_tw[:, :, :, k])

            # --- min over w, onehot, best-slot gather ---
            minv = wp.tile([P, T * M], F32, tag="minv")
            minv_v = minv[:].rearrange("p (t m) -> p t m", t=T)
            nc.vector.tensor_reduce(minv_v, tm_v, mybir.AxisListType.X, OP.min)
            oh = wp.tile([P, T * M * W], F32, tag="oh")
            oh_v = oh[:].rearrange("p (t m w) -> p t m w", t=T, m=M)
            minv_bc = minv_v.unsqueeze(3).broadcast_to([P, T, M, W])
            nc.vector.tensor_tensor(oh_v, tm_v, minv_bc, OP.is_le)

            sb = wp.tile([P, T * 2 * M], F32, tag="sb")  # [t,k,m]
            sb_v = sb[:].rearrange("p (t k m) -> p t k m", t=T, k=2)
            for k in range(2):
                pk = wp.tile([P, T * M * W], F32, tag=f"pk{k}")
                pk_v = pk[:].rearrange("p (t m w) -> p t m w", t=T, m=M)
                s_k = sx_v[:, :, k, :].unsqueeze(2).broadcast_to([P, T, M, W])
                nc.vector.tensor_tensor(pk_v, oh_v, s_k, OP.mult)
                nc.vector.tensor_reduce(
                    sb_v[:, :, k, :], pk_v, mybir.AxisListType.X, OP.add
                )

            # --- conc2x[t,l,m] = sum_k C2[l,k]*sb[k] + d2[l] ---
            conc = wp.tile([P, T * 2 * M], F32, tag="conc")  # [t,l,m]
            conc_v = conc[:].rearrange("p (t l m) -> p t l m", t=T, l=2)
            for l in range(2):
                x0 = wp.tile([P, T * M], F32, tag="x0")
                x0_v = x0[:].rearrange("p (t m) -> p t m", t=T)
                nc.vector.tensor_tensor(
                    x0_v, sb_v[:, :, 0, :], bc_m(ct(3 + 2 * l)), OP.mult
                )
                y0 = wp.tile([P, T * M], F32, tag="y0")
                y0_v = y0[:].rearrange("p (t m) -> p t m", t=T)
                nc.vector.tensor_tensor(
                    y0_v, sb_v[:, :, 1, :], bc_m(ct(4 + 2 * l)), OP.mult
                )
                nc.vector.tensor_tensor(x0_v, x0_v, y0_v, OP.add)
                nc.vector.tensor_tensor(
                    conc_v[:, :, l, :], x0_v, bc_m(ct(7 + l)), OP.add
                )

            # --- A3[t,m] = |conc|^2 + minm + K0' ---
            cq = wp.tile([P, T * 2 * M], F32, tag="cq")
            nc.scalar.activation(
                cq[:], conc[:], mybir.ActivationFunctionType.Square,
                bias=zero_col[:], scale=0.5,
            )
            cq_v = cq[:].rearrange("p (t l m) -> p t l m", t=T, l=2)
            a3 = wp.tile([P, T * M], F32, tag="a3")
            a3_v = a3[:].rearrange("p (t m) -> p t m", t=T)
            nc.vector.tensor_tensor(a3_v, cq_v[:, :, 0, :], cq_v[:, :, 1, :], OP.add)
            nc.vector.tensor_tensor(a3[:], a3[:], minv[:], OP.add)
            nc.vector.tensor_tensor(a3_v, a3_v, bc_m(ct(2)), OP.add)


            # --- |s_w|^2 ---
            ss = wp.tile([P, T * W * L], F32, tag="ss")
            nc.scalar.activation(ss[:], s_t[:], mybir.ActivationFunctionType.Square, bias=zero_col[:])
            ss_v = ss[:].rearrange("p (t w l) -> p t w l", t=T, w=W)
            s2s = wp.tile([P, T * W], F32, tag="s2s")
            s2s_v = s2s[:].rearrange("p (t w) -> p t w", t=T)
            nc.vector.tensor_tensor(
                s2s_v, ss_v[:, :, :, 0], ss_v[:, :, :, 1], OP.add
            )

            # --- E[t,w,m] = conc2x.s - A3 - |s|^2 ;  score = exp(E) ---
            m1e = wp.tile([P, T * W * M], F32, tag="m1")
            m1e_v = m1e[:].rearrange("p (t w m) -> p t w m", t=T, w=W)
            conc0_bc = conc_v[:, :, 0, :].unsqueeze(2).broadcast_to([P, T, W, M])
            s0_bc = sx_v[:, :, 0, :].unsqueeze(3).broadcast_to([P, T, W, M])
            nc.vector.tensor_tensor(m1e_v, conc0_bc, s0_bc, OP.mult)
            m2e = wp.tile([P, T * W * M], F32, tag="m2")
            m2e_v = m2e[:].rearrange("p (t w m) -> p t w m", t=T, w=W)
            conc1_bc = conc_v[:, :, 1, :].unsqueeze(2).broadcast_to([P, T, W, M])
            s1_bc = sx_v[:, :, 1, :].unsqueeze(3).broadcast_to([P, T, W, M])
            nc.vector.tensor_tensor(m2e_v, conc1_bc, s1_bc, OP.mult)

            a3_bc = a3_v.unsqueeze(2).broadcast_to([P, T, W, M])
            nc.vector.tensor_tensor(m1e_v, m1e_v, a3_bc, OP.subtract)
            s2s_bc = s2s_v.unsqueeze(3).broadcast_to([P, T, W, M])
            nc.vector.tensor_tensor(m2e_v, m2e_v, s2s_bc, OP.subtract)
            ee = wp.tile([P, T * W * M], F32, tag="tm")
            nc.vector.tensor_tensor(ee[:], m1e[:], m2e[:], OP.add)
            sc = wp.tile([P, T * W * M], F32, tag="oh")
            nc.scalar.activation(sc[:], ee[:], mybir.ActivationFunctionType.Exp, bias=zero_col[:])

            # --- out[t,w] = sum_m score ---
            ov = op_.tile([P, T * W], F32, tag="ov")
            ov_v = ov[:].rearrange("p (t w) -> p t w", t=T)
            sc_v = sc[:].rearrange("p (t w m) -> p t w m", t=T, w=W)
            nc.vector.tensor_reduce(ov_v, sc_v, mybir.AxisListType.X, OP.add)
            nc.sync.dma_start(out_t[n], ov[:])

    nc.compile()
    return nc


_NC_CACHE = None


def kernel(state, constants, gammas, head_W, head_b, tail_W, tail_b):
    global LAST_EXEC_NS, _NC_CACHE
    state = np.ascontiguousarray(np.asarray(state, dtype=np.float32))
    rtab128, ctab128 = _tables(
        np.asarray(constants), np.asarray(gammas), np.asarray(head_W),
        np.asarray(head_b), np.asarray(tail_W), np.asarray(tail_b),
    )
    if _NC_CACHE is None:
        _NC_CACHE = _build()
    nc = _NC_CACHE
    tabs128 = np.ascontiguousarray(np.concatenate([rtab128, ctab128], axis=1))
    in_maps = []
    for i in range(NCORES):
        in_maps.append(
            {
                "state": state[i * CB : (i + 1) * CB],
                "tabs": tabs128,
            }
        )
    res = run_bass_kernel_spmd(nc, in_maps, core_ids=list(range(NCORES)), trace=TRACE)
    LAST_EXEC_NS = res.exec_time_ns
    out = np.concatenate([res.results[i]["out"] for i in range(NCORES)], axis=0)
    return out


# revision 30
# speedup vs baseline: 1.2568x; 1.0481x over previous
"""Trainium2 Bass kernel for the AlgelogicNetwork problem.

Math (per batch element b, all rule params tiny):
  s[w,l]   : 9 WM slots, 2-dim tokens (state row reshaped [9,2])
  tm[m,w]  = K0'[m] + sum_l Q[m,l]*(s[w,l]+r[m,l])^2     (completed square)
  minm[m]  = min_w tm[m,w];  onehot[m,w] = (tm==minm)
  sb[m,l]  = sum_w onehot[m,w]*s[w,l]                     (best-slot gather)
  conc2x[m,l] = sum_k 2*C[m,l,k]*sb[m,k] + 2*d[m,l]       (2x conclusion)
  E[m,w]   = conc2x[m,:].s[w,:] - |conc2x/2|^2 - |s[w]|^2 - minm[m] - K0'[m]
           = -|conc-s[w]|^2 - minmatch[m]   (so exp(E) = score*confidence)
  out[w]   = sum_m exp(E[m,w])

Sharding: pure data parallel, batch 262144 split 8 ways (32768/core).
Layout: batch in partitions; each SBUF tile holds T batch-groups of 128.
"""

import os
import sys

import numpy as np

if "/opt/trn_rl_repo" not in sys.path:
    sys.path.insert(0, "/opt/trn_rl_repo")

import concourse.bacc as bacc
import concourse.bass as bass
import concourse.tile as tile
from concourse import mybir
from concourse.bass_utils import run_bass_kernel_spmd

F32 = mybir.dt.float32
OP = mybir.AluOpType

M, J, NI, L, W = 16, 2, 3, 2, 9
B = 262144
NCORES = 8
CB = B // NCORES  # 32768 per core
P = 128
T = 16  # batch groups per partition per tile
TILE_B = P * T  # 1024
NTILES = CB // TILE_B  # 32

TRACE = False
LAST_EXEC_NS = None


def _tables(constants, gammas, head_W, head_b, tail_W, tail_b):
    """Host-side folding of the tiny rule params into kernel coefficient tables."""
    f8 = np.float64
    g = 1.0 / (1.0 + np.exp(-gammas[:, :J].astype(f8)))  # [M,J,L]
    c = constants[:, :J].astype(f8)
    omg = 1.0 - g
    Q = omg.sum(1)  # [M,L]
    Pc = (omg * c).sum(1)  # [M,L]
    K2 = (omg * c * c).sum(1)  # [M,L]
    r = -Pc / Q  # [M,L]
    K0p = (K2 - Pc * Pc / Q).sum(1)  # [M]
    A = np.einsum("mjil,mjl->mil", head_W.astype(f8), g)  # [M,I,L]
    bias = np.einsum("mj,mji->mi", g.sum(2), head_b.astype(f8))  # [M,I]
    C = np.einsum("mli,mik->mlk", tail_W.astype(f8), A)  # [M,L,L]
    d = np.einsum("mli,mi->ml", tail_W.astype(f8), bias) + tail_b.astype(f8)  # [M,L]
    C2 = 2.0 * C
    d2 = 2.0 * d

    # rtab: [M,W,L] broadcast of r over w, flattened [288]
    rexp = np.broadcast_to(r[:, None, :], (M, W, L)).reshape(M * W * L)
    # ctab: 9 16-wide tables: q0,q1,k0p,c2_00,c2_01,c2_10,c2_11,d2_0,d2_1
    ctab = np.concatenate(
        [
            Q[:, 0], Q[:, 1], K0p,
            C2[:, 0, 0], C2[:, 0, 1], C2[:, 1, 0], C2[:, 1, 1],
            d2[:, 0], d2[:, 1],
        ]
    )
    rtab128 = np.ascontiguousarray(
        np.broadcast_to(rexp[None].astype(np.float32), (P, rexp.size))
    )
    ctab128 = np.ascontiguousarray(
        np.broadcast_to(ctab[None].astype(np.float32), (P, ctab.size))
    )
    return rtab128, ctab128


def _build():
    nc = bacc.Bacc()
    state = nc.declare_dram_parameter("state", [CB, W * L], F32, isOutput=False)
    tabs = nc.declare_dram_parameter(
        "tabs", [P, M * W * L + 9 * M], F32, isOutput=False
    )
    out = nc.declare_dram_parameter("out", [CB, W], F32, isOutput=True)

    state_t = state[:].rearrange("(n p t) d -> n p (t d)", p=P, t=T)  # [NTILES,128,T*18]
    out_t = out[:].rearrange("(n p t) w -> n p (t w)", p=P, t=T)  # [NTILES,128,T*9]

    from contextlib import ExitStack

    with tile.TileContext(nc) as tc, ExitStack() as ctx:
        cpool = ctx.enter_context(tc.tile_pool(name="consts", bufs=1))
        tabs_t = cpool.tile([P, M * W * L + 9 * M], F32)
        nc.sync.dma_start(tabs_t[:], tabs[:])
        rtab_t = tabs_t[:, : M * W * L]
        ctab_t = tabs_t[:, M * W * L :]
        # gate: absorb the const-DMA wait on DVE once, so per-tile TensorTensor
        # ops never need a second sync-wait slot (TT encoding has only one)
        gate = cpool.tile([P, 1], F32)
        nc.vector.tensor_copy(gate[:], tabs_t[:, :1])
        # explicit zero bias column: keeps Activations off the shared const-ap
        # tile, whose extra semaphore wait overflows the 2-slot AC encoding
        zero_col = cpool.tile([P, 1], F32)
        nc.vector.tensor_tensor(zero_col[:], gate[:], gate[:], OP.subtract)

        def ct(i):  # i-th [128,16] const table
            return ctab_t[:, M * i : M * (i + 1)]

        def bc_m(ap_16):  # [128,16] -> [128,T,16] broadcast over t
            return ap_16.unsqueeze(1).broadcast_to([P, T, M])

        def bc_mw(ap_16):  # [128,16] -> [128,T,16,9]
            return ap_16.unsqueeze(1).unsqueeze(3).broadcast_to([P, T, M, W])

        sp = ctx.enter_context(tc.tile_pool(name="s", bufs=3))
        zp = ctx.enter_context(tc.tile_pool(name="z", bufs=1))
        z2p = ctx.enter_context(tc.tile_pool(name="z2", bufs=2))
        wp = ctx.enter_context(tc.tile_pool(name="work", bufs=2))
        # one slot per tile: output tiles are never reused, so the DVE reduce
        # that writes them never waits on an out-DMA queue release (each
        # compute instruction only has one sync-wait slot)
        op_ = ctx.enter_context(tc.tile_pool(name="outp", bufs=NTILES))

        for n in range(NTILES):
            s_t = sp.tile([P, T * W * L], F32, tag="s")
            nc.sync.dma_start(s_t[:], state_t[n])
            s_tw = s_t[:].rearrange("p (t w l) -> p t w l", t=T, w=W)  # views
            s_tmd = (
                s_t[:].rearrange("p (t d) -> p t d", t=T)
                .unsqueeze(2)
                .broadcast_to([P, T, M, W * L])
            )

            # --- tm[m,w] = sum_l Q_l*(s_wl + r_ml)^2  (K0' folded later) ---
            z = zp.tile([P, T * M * W * L], F32, tag="z")
            z_v = z[:].rearrange("p (t m d) -> p t m d", t=T, m=M)
            r_bc = (
                rtab_t.rearrange("p (m d) -> p m d", m=M)
                .unsqueeze(1)
                .broadcast_to([P, T, M, W * L])
            )
            nc.vector.tensor_tensor(z_v, s_tmd, r_bc, OP.add)
            z2 = z2p.tile([P, T * M * W * L], F32, tag="z2")
            nc.scalar.activation(z2[:], z[:], mybir.ActivationFunctionType.Square, bias=zero_col[:])
            z2_v = z2[:].rearrange("p (t m w l) -> p t m w l", t=T, m=M, w=W)
            m1 = wp.tile([P, T * M * W], F32, tag="m1")
            m1_v = m1[:].rearrange("p (t m w) -> p t m w", t=T, m=M)
            nc.gpsimd.tensor_tensor(m1_v, z2_v[:, :, :, :, 0], bc_mw(ct(0)), OP.mult)
            m2 = wp.tile([P, T * M * W], F32, tag="m2")
            m2_v = m2[:].rearrange("p (t m w) -> p t m w", t=T, m=M)
            nc.gpsimd.tensor_tensor(m2_v, z2_v[:, :, :, :, 1], bc_mw(ct(1)), OP.mult)
            tm = wp.tile([P, T * M * W], F32, tag="tm")
            nc.vector.tensor_tensor(tm[:], m1[:], m2[:], OP.add)
            tm_v = tm[:].rearrange("p (t m w) -> p t m w", t=T, m=M)

            # contiguous per-component slot values: avoids stride-2 operand
            # APs on the four big DVE products below (measured ~40% slower)
            sx = wp.tile([P, T * 2 * W], F32, tag="sx")  # [t,k,w]
            sx_v = sx[:].rearrange("p (t k w) -> p t k w", t=T, k=2)
            for k in range(2):
                nc.scalar.copy(sx_v[:, :, k, :], s_tw[:, :, :, k])

            # --- min over w, onehot, best-slot gather ---
            minv = wp.tile([P, T * M], F32, tag="minv")
            minv_v = minv[:].rearrange("p (t m) -> p t m", t=T)
            nc.vector.tensor_reduce(minv_v, tm_v, mybir.AxisListType.X, OP.min)
            oh = wp.tile([P, T * M * W], F32, tag="oh")
            oh_v = oh[:].rearrange("p (t m w) -> p t m w", t=T, m=M)
            minv_bc = minv_v.unsqueeze(3).broadcast_to([P, T, M, W])
            nc.vector.tensor_tensor(oh_v, tm_v, minv_bc, OP.is_le)

            sb = wp.tile([P, T * 2 * M], F32, tag="sb")  # [t,k,m]
            sb_v = sb[:].rearrange("p (t k m) -> p t k m", t=T, k=2)
            for k in range(2):
                pk = wp.tile([P, T * M * W], F32, tag=f"pk{k}")
                pk_v = pk[:].rearrange("p (t m w) -> p t m w", t=T, m=M)
                s_k = sx_v[:, :, k, :].unsqueeze(2).broadcast_to([P, T, M, W])
                nc.vector.tensor_tensor(pk_v, oh_v, s_k, OP.mult)
                nc.vector.tensor_reduce(
                    sb_v[:, :, k, :], pk_v, mybir.AxisListType.X, OP.add
                )

            # --- conc2x[t,l,m] = sum_k C2[l,k]*sb[k] + d2[l] ---
            conc = wp.tile([P, T * 2 * M], F32, tag="conc")  # [t,l,m]
            conc_v = conc[:].rearrange("p (t l m) -> p t l m", t=T, l=2)
            for l in range(2):
                x0 = wp.tile([P, T * M], F32, tag="x0")
                x0_v = x0[:].rearrange("p (t m) -> p t m", t=T)
                nc.vector.tensor_tensor(
                    x0_v, sb_v[:, :, 0, :], bc_m(ct(3 + 2 * l)), OP.mult
                )
                y0 = wp.tile([P, T * M], F32, tag="y0")
                y0_v = y0[:].rearrange("p (t m) -> p t m", t=T)
                nc.vector.tensor_tensor(
                    y0_v, sb_v[:, :, 1, :], bc_m(ct(4 + 2 * l)), OP.mult
                )
                nc.vector.tensor_tensor(x0_v, x0_v, y0_v, OP.add)
                nc.vector.tensor_tensor(
                    conc_v[:, :, l, :], x0_v, bc_m(ct(7 + l)), OP.add
                )

            # --- A3[t,m] = |conc|^2 + minm + K0' ---
            cq = wp.tile([P, T * 2 * M], F32, tag="cq")
            nc.scalar.activation(
                cq[:], conc[:], mybir.ActivationFunctionType.Square,
                bias=zero_col[:], scale=0.5,
            )
            cq_v = cq[:].rearrange("p (t l m) -> p t l m", t=T, l=2)
            a3 = wp.tile([P, T * M], F32, tag="a3")
            a3_v = a3[:].rearrange("p (t m) -> p t m", t=T)
            nc.vector.tensor_tensor(a3_v, cq_v[:, :, 0, :], cq_v[:, :, 1, :], OP.add)
            nc.vector.tensor_tensor(a3[:], a3[:], minv[:], OP.add)
            nc.vector.tensor_tensor(a3_v, a3_v, bc_m(ct(2)), OP.add)


            # --- |s_w|^2 ---
            ss = wp.tile([P, T * W * L], F32, tag="ss")
            nc.scalar.activation(ss[:], s_t[:], mybir.ActivationFunctionType.Square, bias=zero_col[:])
            ss_v = ss[:].rearrange("p (t w l) -> p t w l", t=T, w=W)
            s2s = wp.tile([P, T * W], F32, tag="s2s")
            s2s_v = s2s[:].rearrange("p (t w) -> p t w", t=T)
            nc.vector.tensor_tensor(
                s2s_v, ss_v[:, :, :, 0], ss_v[:, :, :, 1], OP.add
            )
            exps2 = wp.tile([P, T * W], F32, tag="exps2")
            nc.scalar.activation(
                exps2[:], s2s[:], mybir.ActivationFunctionType.Exp,
                bias=zero_col[:], scale=-1.0,
            )

            # --- E[t,w,m] = conc2x.s - A3 ;  score = exp(E) ---
            m1e = wp.tile([P, T * W * M], F32, tag="m1")
            m1e_v = m1e[:].rearrange("p (t w m) -> p t w m", t=T, w=W)
            conc0_bc = conc_v[:, :, 0, :].unsqueeze(2).broadcast_to([P, T, W, M])
            s0_bc = sx_v[:, :, 0, :].unsqueeze(3).broadcast_to([P, T, W, M])
            nc.vector.tensor_tensor(m1e_v, conc0_bc, s0_bc, OP.mult)
            m2e = wp.tile([P, T * W * M], F32, tag="m2")
            m2e_v = m2e[:].rearrange("p (t w m) -> p t w m", t=T, w=W)
            conc1_bc = conc_v[:, :, 1, :].unsqueeze(2).broadcast_to([P, T, W, M])
            s1_bc = sx_v[:, :, 1, :].unsqueeze(3).broadcast_to([P, T, W, M])
            nc.vector.tensor_tensor(m2e_v, conc1_bc, s1_bc, OP.mult)

            a3_bc = a3_v.unsqueeze(2).broadcast_to([P, T, W, M])
            nc.vector.tensor_tensor(m1e_v, m1e_v, a3_bc, OP.subtract)
            ee = wp.tile([P, T * W * M], F32, tag="tm")
            nc.vector.tensor_tensor(ee[:], m1e[:], m2e[:], OP.add)
            sc = wp.tile([P, T * W * M], F32, tag="oh")
            nc.scalar.activation(sc[:], ee[:], mybir.ActivationFunctionType.Exp, bias=zero_col[:])

            # --- out[t,w] = sum_m score ---
            ovm = wp.tile([P, T * W], F32, tag="ovm")
            ovm_v = ovm[:].rearrange("p (t w) -> p t w", t=T)
            sc_v = sc[:].rearrange("p (t w m) -> p t w m", t=T, w=W)
            nc.vector.tensor_reduce(ovm_v, sc_v, mybir.AxisListType.X, OP.add)
            ov = op_.tile([P, T * W], F32, tag="ov")
            nc.vector.tensor_tensor(ov[:], ovm[:], exps2[:], OP.mult)
            nc.sync.dma_start(out_t[n], ov[:])

    nc.compile()
    return nc


_NC_CACHE = None


def kernel(state, constants, gammas, head_W, head_b, tail_W, tail_b):
    global LAST_EXEC_NS, _NC_CACHE
    state = np.ascontiguousarray(np.asarray(state, dtype=np.float32))
    rtab128, ctab128 = _tables(
        np.asarray(constants), np.asarray(gammas), np.asarray(head_W),
        np.asarray(head_b), np.asarray(tail_W), np.asarray(tail_b),
    )
    if _NC_CACHE is None:
        _NC_CACHE = _build()
    nc = _NC_CACHE
    tabs128 = np.ascontiguousarray(np.concatenate([rtab128, ctab128], axis=1))
    in_maps = []
    for i in range(NCORES):
        in_maps.append(
            {
                "state": state[i * CB : (i + 1) * CB],
                "tabs": tabs128,
            }
        )
    res = run_bass_kernel_spmd(nc, in_maps, core_ids=list(range(NCORES)), trace=TRACE)
    LAST_EXEC_NS = res.exec_time_ns
    out = np.concatenate([res.results[i]["out"] for i in range(NCORES)], axis=0)
    return out


# revision 31
# speedup vs baseline: 1.2571x; 1.0002x over previous
"""Trainium2 Bass kernel for the AlgelogicNetwork problem.

Math (per batch element b, all rule params tiny):
  s[w,l]   : 9 WM slots, 2-dim tokens (state row reshaped [9,2])
  tm[m,w]  = K0'[m] + sum_l Q[m,l]*(s[w,l]+r[m,l])^2     (completed square)
  minm[m]  = min_w tm[m,w];  onehot[m,w] = (tm==minm)
  sb[m,l]  = sum_w onehot[m,w]*s[w,l]                     (best-slot gather)
  conc2x[m,l] = sum_k 2*C[m,l,k]*sb[m,k] + 2*d[m,l]       (2x conclusion)
  E[m,w]   = conc2x[m,:].s[w,:] - |conc2x/2|^2 - |s[w]|^2 - minm[m] - K0'[m]
           = -|conc-s[w]|^2 - minmatch[m]   (so exp(E) = score*confidence)
  out[w]   = sum_m exp(E[m,w])

Sharding: pure data parallel, batch 262144 split 8 ways (32768/core).
Layout: batch in partitions; each SBUF tile holds T batch-groups of 128.
"""

import os
import sys

import numpy as np

if "/opt/trn_rl_repo" not in sys.path:
    sys.path.insert(0, "/opt/trn_rl_repo")

import concourse.bacc as bacc
import concourse.bass as bass
import concourse.tile as tile
from concourse import mybir
from concourse.bass_utils import run_bass_kernel_spmd

F32 = mybir.dt.float32
OP = mybir.AluOpType

M, J, NI, L, W = 16, 2, 3, 2, 9
B = 262144
NCORES = 8
CB = B // NCORES  # 32768 per core
P = 128
T = 16  # batch groups per partition per tile
TILE_B = P * T  # 1024
NTILES = CB // TILE_B  # 32

TRACE = False
LAST_EXEC_NS = None


def _tables(constants, gammas, head_W, head_b, tail_W, tail_b):
    """Host-side folding of the tiny rule params into kernel coefficient tables."""
    f8 = np.float64
    g = 1.0 / (1.0 + np.exp(-gammas[:, :J].astype(f8)))  # [M,J,L]
    c = constants[:, :J].astype(f8)
    omg = 1.0 - g
    Q = omg.sum(1)  # [M,L]
    Pc = (omg * c).sum(1)  # [M,L]
    K2 = (omg * c * c).sum(1)  # [M,L]
    r = -Pc / Q  # [M,L]
    K0p = (K2 - Pc * Pc / Q).sum(1)  # [M]
    A = np.einsum("mjil,mjl->mil", head_W.astype(f8), g)  # [M,I,L]
    bias = np.einsum("mj,mji->mi", g.sum(2), head_b.astype(f8))  # [M,I]
    C = np.einsum("mli,mik->mlk", tail_W.astype(f8), A)  # [M,L,L]
    d = np.einsum("mli,mi->ml", tail_W.astype(f8), bias) + tail_b.astype(f8)  # [M,L]
    C2 = 2.0 * C
    d2 = 2.0 * d

    # rtab: [M,W,L] broadcast of r over w, flattened [288]
    rexp = np.broadcast_to(r[:, None, :], (M, W, L)).reshape(M * W * L)
    # ctab: 9 16-wide tables: q0,q1,k0p,c2_00,c2_01,c2_10,c2_11,d2_0,d2_1
    ctab = np.concatenate(
        [
            Q[:, 0], Q[:, 1], K0p,
            C2[:, 0, 0], C2[:, 0, 1], C2[:, 1, 0], C2[:, 1, 1],
            d2[:, 0], d2[:, 1],
        ]
    )
    rtab128 = np.ascontiguousarray(
        np.broadcast_to(rexp[None].astype(np.float32), (P, rexp.size))
    )
    ctab128 = np.ascontiguousarray(
        np.broadcast_to(ctab[None].astype(np.float32), (P, ctab.size))
    )
    return rtab128, ctab128


def _build():
    nc = bacc.Bacc()
    state = nc.declare_dram_parameter("state", [CB, W * L], F32, isOutput=False)
    tabs = nc.declare_dram_parameter(
        "tabs", [P, M * W * L + 9 * M], F32, isOutput=False
    )
    out = nc.declare_dram_parameter("out", [CB, W], F32, isOutput=True)

    state_t = state[:].rearrange("(n p t) d -> n p (t d)", p=P, t=T)  # [NTILES,128,T*18]
    out_t = out[:].rearrange("(n p t) w -> n p (t w)", p=P, t=T)  # [NTILES,128,T*9]

    from contextlib import ExitStack

    with tile.TileContext(nc) as tc, ExitStack() as ctx:
        cpool = ctx.enter_context(tc.tile_pool(name="consts", bufs=1))
        tabs_t = cpool.tile([P, M * W * L + 9 * M], F32)
        nc.sync.dma_start(tabs_t[:], tabs[:])
        rtab_t = tabs_t[:, : M * W * L]
        ctab_t = tabs_t[:, M * W * L :]
        # gate: absorb the const-DMA wait on DVE once, so per-tile TensorTensor
        # ops never need a second sync-wait slot (TT encoding has only one)
        gate = cpool.tile([P, 1], F32)
        nc.vector.tensor_copy(gate[:], tabs_t[:, :1])
        # explicit zero bias column: keeps Activations off the shared const-ap
        # tile, whose extra semaphore wait overflows the 2-slot AC encoding
        zero_col = cpool.tile([P, 1], F32)
        nc.vector.tensor_tensor(zero_col[:], gate[:], gate[:], OP.subtract)

        def ct(i):  # i-th [128,16] const table
            return ctab_t[:, M * i : M * (i + 1)]

        def bc_m(ap_16):  # [128,16] -> [128,T,16] broadcast over t
            return ap_16.unsqueeze(1).broadcast_to([P, T, M])

        def bc_mw(ap_16):  # [128,16] -> [128,T,16,9]
            return ap_16.unsqueeze(1).unsqueeze(3).broadcast_to([P, T, M, W])

        sp = ctx.enter_context(tc.tile_pool(name="s", bufs=3))
        zp = ctx.enter_context(tc.tile_pool(name="z", bufs=1))
        z2p = ctx.enter_context(tc.tile_pool(name="z2", bufs=2))
        wp = ctx.enter_context(tc.tile_pool(name="work", bufs=2))
        # one slot per tile: output tiles are never reused, so the DVE reduce
        # that writes them never waits on an out-DMA queue release (each
        # compute instruction only has one sync-wait slot)
        op_ = ctx.enter_context(tc.tile_pool(name="outp", bufs=NTILES))

        for n in range(NTILES):
            s_t = sp.tile([P, T * W * L], F32, tag="s")
            nc.sync.dma_start(s_t[:], state_t[n])
            s_tw = s_t[:].rearrange("p (t w l) -> p t w l", t=T, w=W)  # views
            s_tmd = (
                s_t[:].rearrange("p (t d) -> p t d", t=T)
                .unsqueeze(2)
                .broadcast_to([P, T, M, W * L])
            )

            # --- tm[m,w] = sum_l Q_l*(s_wl + r_ml)^2  (K0' folded later) ---
            z = zp.tile([P, T * M * W * L], F32, tag="z")
            z_v = z[:].rearrange("p (t m d) -> p t m d", t=T, m=M)
            r_bc = (
                rtab_t.rearrange("p (m d) -> p m d", m=M)
                .unsqueeze(1)
                .broadcast_to([P, T, M, W * L])
            )
            nc.vector.tensor_tensor(z_v, s_tmd, r_bc, OP.add)
            z2 = z2p.tile([P, T * M * W * L], F32, tag="z2")
            nc.scalar.activation(z2[:], z[:], mybir.ActivationFunctionType.Square, bias=zero_col[:])
            z2_v = z2[:].rearrange("p (t m w l) -> p t m w l", t=T, m=M, w=W)
            m1 = wp.tile([P, T * M * W], F32, tag="m1")
            m1_v = m1[:].rearrange("p (t m w) -> p t m w", t=T, m=M)
            nc.gpsimd.tensor_tensor(m1_v, z2_v[:, :, :, :, 0], bc_mw(ct(0)), OP.mult)
            m2 = wp.tile([P, T * M * W], F32, tag="m2")
            m2_v = m2[:].rearrange("p (t m w) -> p t m w", t=T, m=M)
            nc.gpsimd.tensor_tensor(m2_v, z2_v[:, :, :, :, 1], bc_mw(ct(1)), OP.mult)
            tm = wp.tile([P, T * M * W], F32, tag="tm")
            nc.vector.tensor_tensor(tm[:], m1[:], m2[:], OP.add)
            tm_v = tm[:].rearrange("p (t m w) -> p t m w", t=T, m=M)

            # contiguous per-component slot values: avoids stride-2 operand
            # APs on the four big DVE products below (measured ~40% slower)
            sx = wp.tile([P, T * 2 * W], F32, tag="sx")  # [t,k,w]
            sx_v = sx[:].rearrange("p (t k w) -> p t k w", t=T, k=2)
            for k in range(2):
                nc.scalar.copy(sx_v[:, :, k, :], s_tw[:, :, :, k])

            # --- min over w, onehot, best-slot gather ---
            minv = wp.tile([P, T * M], F32, tag="minv")
            minv_v = minv[:].rearrange("p (t m) -> p t m", t=T)
            nc.vector.tensor_reduce(minv_v, tm_v, mybir.AxisListType.X, OP.min)
            oh = wp.tile([P, T * M * W], F32, tag="oh")
            oh_v = oh[:].rearrange("p (t m w) -> p t m w", t=T, m=M)
            minv_bc = minv_v.unsqueeze(3).broadcast_to([P, T, M, W])
            nc.vector.tensor_tensor(oh_v, tm_v, minv_bc, OP.is_le)

            sb = wp.tile([P, T * 2 * M], F32, tag="sb")  # [t,k,m]
            sb_v = sb[:].rearrange("p (t k m) -> p t k m", t=T, k=2)
            for k in range(2):
                pk = wp.tile([P, T * M * W], F32, tag=f"pk{k}")
                pk_v = pk[:].rearrange("p (t m w) -> p t m w", t=T, m=M)
                s_k = sx_v[:, :, k, :].unsqueeze(2).broadcast_to([P, T, M, W])
                nc.vector.tensor_tensor(pk_v, oh_v, s_k, OP.mult)
                nc.vector.tensor_reduce(
                    sb_v[:, :, k, :], pk_v, mybir.AxisListType.X, OP.add
                )

            # --- conc2x[t,l,m] = sum_k C2[l,k]*sb[k] + d2[l] ---
            conc = wp.tile([P, T * 2 * M], F32, tag="conc")  # [t,l,m]
            conc_v = conc[:].rearrange("p (t l m) -> p t l m", t=T, l=2)
            for l in range(2):
                x0 = wp.tile([P, T * M], F32, tag="x0")
                x0_v = x0[:].rearrange("p (t m) -> p t m", t=T)
                nc.vector.tensor_tensor(
                    x0_v, sb_v[:, :, 0, :], bc_m(ct(3 + 2 * l)), OP.mult
                )
                y0 = wp.tile([P, T * M], F32, tag="y0")
                y0_v = y0[:].rearrange("p (t m) -> p t m", t=T)
                nc.vector.tensor_tensor(
                    y0_v, sb_v[:, :, 1, :], bc_m(ct(4 + 2 * l)), OP.mult
                )
                nc.vector.tensor_tensor(x0_v, x0_v, y0_v, OP.add)
                nc.vector.tensor_tensor(
                    conc_v[:, :, l, :], x0_v, bc_m(ct(7 + l)), OP.add
                )

            # --- A3[t,m] = |conc|^2 + minm + K0' ---
            cq = wp.tile([P, T * 2 * M], F32, tag="cq")
            nc.scalar.activation(
                cq[:], conc[:], mybir.ActivationFunctionType.Square,
                bias=zero_col[:], scale=0.5,
            )
            cq_v = cq[:].rearrange("p (t l m) -> p t l m", t=T, l=2)
            a3 = wp.tile([P, T * M], F32, tag="a3")
            a3_v = a3[:].rearrange("p (t m) -> p t m", t=T)
            nc.vector.tensor_tensor(a3_v, cq_v[:, :, 0, :], cq_v[:, :, 1, :], OP.add)
            nc.vector.tensor_tensor(a3[:], a3[:], minv[:], OP.add)
            nc.vector.tensor_tensor(a3_v, a3_v, bc_m(ct(2)), OP.add)


            # --- |s_w|^2 ---
            ss = wp.tile([P, T * W * L], F32, tag="ss")
            nc.scalar.activation(ss[:], s_t[:], mybir.ActivationFunctionType.Square, bias=zero_col[:])
            ss_v = ss[:].rearrange("p (t w l) -> p t w l", t=T, w=W)
            s2s = wp.tile([P, T * W], F32, tag="s2s")
            s2s_v = s2s[:].rearrange("p (t w) -> p t w", t=T)
            nc.vector.tensor_tensor(
                s2s_v, ss_v[:, :, :, 0], ss_v[:, :, :, 1], OP.add
            )
            exps2 = wp.tile([P, T * W], F32, tag="exps2")
            nc.scalar.activation(
                exps2[:], s2s[:], mybir.ActivationFunctionType.Exp,
                bias=zero_col[:], scale=-1.0,
            )

            # --- E[t,w,m] = conc2x.s - A3 ;  score = exp(E) ---
            m1e = wp.tile([P, T * W * M], F32, tag="m1")
            m1e_v = m1e[:].rearrange("p (t w m) -> p t w m", t=T, w=W)
            conc0_bc = conc_v[:, :, 0, :].unsqueeze(2).broadcast_to([P, T, W, M])
            s0_bc = sx_v[:, :, 0, :].unsqueeze(3).broadcast_to([P, T, W, M])
            nc.vector.tensor_tensor(m1e_v, conc0_bc, s0_bc, OP.mult)
            m2e = wp.tile([P, T * W * M], F32, tag="m2")
            m2e_v = m2e[:].rearrange("p (t w m) -> p t w m", t=T, w=W)
            conc1_bc = conc_v[:, :, 1, :].unsqueeze(2).broadcast_to([P, T, W, M])
            s1_bc = sx_v[:, :, 1, :].unsqueeze(3).broadcast_to([P, T, W, M])
            nc.vector.tensor_tensor(m2e_v, conc1_bc, s1_bc, OP.mult)

            a3_bc = a3_v.unsqueeze(2).broadcast_to([P, T, W, M])
            nc.vector.tensor_tensor(m1e_v, m1e_v, a3_bc, OP.subtract)
            ee = wp.tile([P, T * W * M], F32, tag="tm")
            nc.vector.tensor_tensor(ee[:], m1e[:], m2e[:], OP.add)
            sc = wp.tile([P, T * W * M], mybir.dt.bfloat16, tag="oh")
            nc.scalar.activation(sc[:], ee[:], mybir.ActivationFunctionType.Exp, bias=zero_col[:])

            # --- out[t,w] = sum_m score ---
            ovm = wp.tile([P, T * W], F32, tag="ovm")
            ovm_v = ovm[:].rearrange("p (t w) -> p t w", t=T)
            sc_v = sc[:].rearrange("p (t w m) -> p t w m", t=T, w=W)
            nc.vector.tensor_reduce(ovm_v, sc_v, mybir.AxisListType.X, OP.add)
            ov = op_.tile([P, T * W], F32, tag="ov")
            nc.vector.tensor_tensor(ov[:], ovm[:], exps2[:], OP.mult)
            nc.sync.dma_start(out_t[n], ov[:])

    nc.compile()
    return nc


_NC_CACHE = None


def kernel(state, constants, gammas, head_W, head_b, tail_W, tail_b):
    global LAST_EXEC_NS, _NC_CACHE
    state = np.ascontiguousarray(np.asarray(state, dtype=np.float32))
    rtab128, ctab128 = _tables(
        np.asarray(constants), np.asarray(gammas), np.asarray(head_W),
        np.asarray(head_b), np.asarray(tail_W), np.asarray(tail_b),
    )
    if _NC_CACHE is None:
        _NC_CACHE = _build()
    nc = _NC_CACHE
    tabs128 = np.ascontiguousarray(np.concatenate([rtab128, ctab128], axis=1))
    in_maps = []
    for i in range(NCORES):
        in_maps.append(
            {
                "state": state[i * CB : (i + 1) * CB],
                "tabs": tabs128,
            }
        )
    res = run_bass_kernel_spmd(nc, in_maps, core_ids=list(range(NCORES)), trace=TRACE)
    LAST_EXEC_NS = res.exec_time_ns
    out = np.concatenate([res.results[i]["out"] for i in range(NCORES)], axis=0)
    return out
